# revision 6
# baseline (speedup 1.0000x reference)
"""3-layer GCN (message passing) on 8 Trainium2 NeuronCores.

Strategy (per spec sharding hint): partition destination nodes (and their
incoming edges) across the 8 cores; replicate the 64x64 weights; the
segment-sum runs locally per dst partition against a full replicated
source-feature table that is rebuilt and all-gathered between layers.

Math: per layer  h' = LR(segsum(w * (h@W)[src])) = LR(segsum(w*h[src]) @ W),
so each layer gathers from a table T_l that already has the weight folded in:
  T_1 = x,  T_{l+1} = (LR-output of layer l) @ W_{l+1}.
Layer l computes agg = segsum(w * T_l[src]); layer-1 applies W1 after the
aggregation (T_1 = x), layers 2/3 use tables with W pre-applied.

Device mapping per core:
  - edges sorted by (dst 128-group, src half); each (group, half) segment
    padded to a 128 multiple; chunk counts made uniform across cores so all
    8 cores run one SPMD program.
  - gather: nc.gpsimd.dma_gather from a bf16 duplicated-row table
    ([n,128]bf16 = 256B rows), int16 indices; two AP views of the table
    (offset 0 / offset nrows-32768) keep indices < 32768. 4 SWDGE queues.
  - scatter: one-hot S [128e x 128d] (bf16, values edge_w) built on DVE via
    iota==dst_local compare; PE matmul S^T @ msg accumulates in PSUM.
  - group epilogue: LeakyReLU on ScalarE; PE transpose + small matmuls build
    the next layer's table slice; AllGather (collective) replicates it.
"""

import os
import sys

sys.path.insert(0, "/opt/trn_rl_repo")

import numpy as np
import ml_dtypes

from concourse import bass, bacc, tile, mybir
from concourse.bass_utils import run_bass_kernel_spmd

NC = 8
D = 64
NEG_SLOPE = 0.01
GROUP = 128            # dst nodes per PSUM accumulation group
MAX_CALL_CHUNKS = 8    # chunks (of 128 idxs) per dma_gather call
NQ = 4                 # SWDGE queues

BF16 = mybir.dt.bfloat16
F32 = mybir.dt.float32
I16 = mybir.dt.int16

LAST_EXEC_NS = None    # set when KERNEL_PROFILE=1


def _install_profile_shim():
    """Provide antenv.axon_hooks (NTFF profiling) if the image lacks it."""
    import types

    if "antenv.axon_hooks" in sys.modules:
        return
    mod = types.ModuleType("antenv.axon_hooks")
    holder = [None]
    mod.set_axon_ntff_profile_hook = lambda h: holder.__setitem__(0, h)
    mod.get_axon_ntff_profile_hook = lambda: holder[0]
    sys.modules["antenv.axon_hooks"] = mod
    try:
        import antenv

        antenv.axon_hooks = mod
    except ImportError:
        pass
    try:
        from trn_agent_boot.trn_boot import _ntff_profile_via_ctypes

        h = _ntff_profile_via_ctypes("/opt/axon/libaxon_pjrt.so")
        if h is not None:
            mod.set_axon_ntff_profile_hook(h)
    except Exception:
        pass


def _prep_edges(src, dst, w, n_nodes, nodes_per_core, tab_rows_l1, tab_rows_l23):
    """Partition/sort/pad edges; build the static per-core chunk schedule.

    Returns (sched, per_core) where sched is the compile-time structure
    (identical for all cores) and per_core holds each core's packed arrays.
    sched: list over groups of dict(half -> n_chunks)
    per_core[c]: dict with idx1, idx23, w, dl arrays of shape [n_chunks_total, 128]
    """
    n_groups = int(np.ceil(nodes_per_core / GROUP))
    # half split threshold on raw src ids, chosen so that BOTH the raw id and
    # the padded-table id (c*rows_pad + r) stay < 32768 for half A, and both
    # B-view offsets stay reachable. 32000 leaves margin on both sides.
    halfA_max = 32000
    offB_l1 = max(0, tab_rows_l1 - 32768)
    offB_l23 = max(0, tab_rows_l23 - 32768)
    rows_pad = n_groups * GROUP  # padded per-core dst rows (6272)

    per_core_edges = []
    counts = np.zeros((NC, n_groups, 2), np.int64)
    for c in range(NC):
        lo, hi = c * nodes_per_core, (c + 1) * nodes_per_core
        m = (dst >= lo) & (dst < hi)
        es, ed, ew = src[m], dst[m] - lo, w[m]
        grp = ed // GROUP
        half = (es >= halfA_max).astype(np.int64)  # 0 = A, 1 = B
        order = np.lexsort((half, grp))
        es, ed, ew, grp, half = es[order], ed[order], ew[order], grp[order], half[order]
        per_core_edges.append((es, ed, ew, grp, half))
        for g in range(n_groups):
            gm = grp == g
            counts[c, g, 0] = int((gm & (half == 0)).sum())
            counts[c, g, 1] = int((gm & (half == 1)).sum())

    # static chunk counts: max over cores, >= 1 chunk for half A of each group
    n_chunks = np.zeros((n_groups, 2), np.int64)
    v_counts = np.zeros((n_groups, 2), np.int64)  # 16-aligned gather counts
    for g in range(n_groups):
        for h in range(2):
            mx = int(counts[:, g, h].max())
            v = max(16, 16 * ((mx + 15) // 16))
            if h == 1 and mx == 0:
                v = 0
            k = (v + GROUP - 1) // GROUP
            n_chunks[g, h] = k
            v_counts[g, h] = v
    total_chunks = int(n_chunks.sum())

    per_core = []
    for c in range(NC):
        es, ed, ew, grp, half = per_core_edges[c]
        idx1 = np.zeros((total_chunks, GROUP), np.int16)
        idx23 = np.zeros((total_chunks, GROUP), np.int16)
        wv = np.zeros((total_chunks, GROUP), np.float32)
        dl = np.zeros((total_chunks, GROUP), np.int16)
        ci = 0
        for g in range(n_groups):
            for h in range(2):
                k = int(n_chunks[g, h])
                if k == 0:
                    continue
                gm = (grp == g) & (half == h)
                s_, d_, w_ = es[gm], ed[gm], ew[gm]
                n = len(s_)
                slots = k * GROUP
                assert n <= slots
                buf_s = np.zeros(slots, np.int64)
                buf_d = np.zeros(slots, np.int64)
                buf_w = np.zeros(slots, np.float32)
                buf_s[:n] = s_
                buf_d[:n] = d_ - g * GROUP
                buf_w[:n] = w_
                if h == 1 and n < slots:
                    buf_s[n:] = halfA_max  # valid row for the B view
                # layer-1 index (raw node id), layer-2/3 index (6272-padded id)
                i1 = np.where(buf_s < halfA_max, buf_s, buf_s - offB_l1)
                s23 = (buf_s // nodes_per_core) * rows_pad + buf_s % nodes_per_core
                # same half split must hold for the remapped ids
                i23 = np.where(buf_s < halfA_max, s23, s23 - offB_l23)
                assert i1.max() < 32768 and i1.min() >= 0
                assert i23.max() < 32768 and i23.min() >= 0
                idx1[ci : ci + k] = i1.reshape(k, GROUP).astype(np.int16)
                idx23[ci : ci + k] = i23.reshape(k, GROUP).astype(np.int16)
                wv[ci : ci + k] = buf_w.reshape(k, GROUP)
                dl[ci : ci + k] = buf_d.reshape(k, GROUP).astype(np.int16)
                ci += k
        assert ci == total_chunks
        per_core.append(dict(idx1=idx1, idx23=idx23, w=wv, dl=dl))

    sched = dict(n_groups=n_groups, n_chunks=n_chunks, v_counts=v_counts,
                 total_chunks=total_chunks,
                 rows_pad=rows_pad, offB_l1=offB_l1, offB_l23=offB_l23)
    return sched, per_core


def _wrap_idx(idx_chunks):
    """[n_chunks,128] int16 -> SBUF wrap layout [128, n_chunks*8].

    dma_gather reads index at stream position p from (partition p%16,
    col p//16), replicated across the 8 q7 core groups (x8 on partitions).
    Calls slice contiguous column ranges, so pack per chunk: chunk i's 128
    positions occupy cols [8i, 8i+8).
    """
    n = idx_chunks.shape[0]
    w16 = idx_chunks.reshape(n * 8, 16).T  # [16, n*8]: pos p of chunk i -> (p%16, 8i+p//16)
    return np.tile(w16, (8, 1)).copy()


def _build_nc(n_nodes, sched, dbg=False):
    nodes_per_core = n_nodes // NC
    n_groups = sched["n_groups"]
    n_chunks = sched["n_chunks"]
    total_chunks = sched["total_chunks"]
    rows_pad = sched["rows_pad"]
    tab_rows = NC * rows_pad

    nc = bacc.Bacc("TRN2", target_bir_lowering=False, debug=False,
                   num_devices=NC, num_swdge_queues=NQ)

    # ---- I/O ----
    xdup_d = nc.dram_tensor("xdup", [n_nodes, 2 * D], BF16, kind="ExternalInput")
    idx1_d = nc.dram_tensor("idx1", [128, total_chunks * 8], I16, kind="ExternalInput")
    idx23_d = nc.dram_tensor("idx23", [128, total_chunks * 8], I16, kind="ExternalInput")
    w_d = nc.dram_tensor("wv", [128, total_chunks], BF16, kind="ExternalInput")
    dl_d = nc.dram_tensor("dl", [128, total_chunks], BF16, kind="ExternalInput")
    iota_d = nc.dram_tensor("iota", [128, 128], BF16, kind="ExternalInput")
    ident_d = nc.dram_tensor("ident", [128, 128], BF16, kind="ExternalInput")
    ws_d = nc.dram_tensor("ws", [3 * D, D], BF16, kind="ExternalInput")  # W1;W2;W3
    out_d = nc.dram_tensor("out", [nodes_per_core, D], F32, kind="ExternalOutput")

    # ---- tables ----
    g2_loc = nc.dram_tensor("g2_loc", [rows_pad, 2 * D], BF16, kind="Internal")
    g3_loc = nc.dram_tensor("g3_loc", [rows_pad, 2 * D], BF16, kind="Internal")
    g2_full = nc.dram_tensor("g2_full", [tab_rows, 2 * D], BF16, kind="Internal",
                             addr_space="Shared")
    g3_full = nc.dram_tensor("g3_full", [tab_rows, 2 * D], BF16, kind="Internal",
                             addr_space="Shared")

    offB_l1 = sched["offB_l1"]
    offB_l23 = sched["offB_l23"]

    with tile.TileContext(nc) as tc:
        with (
            tc.tile_pool(name="res", bufs=1) as res,
            tc.tile_pool(name="gp", bufs=12) as gpool,
            tc.tile_pool(name="sp", bufs=12) as spool,
            tc.tile_pool(name="ep", bufs=6) as epool,
            tc.tile_pool(name="ps_agg", bufs=4, space="PSUM") as ps_agg,
            tc.tile_pool(name="ps_tr", bufs=2, space="PSUM") as ps_tr,
            tc.tile_pool(name="ps_w", bufs=2, space="PSUM") as ps_w,
        ):
            idx1_t = res.tile([128, total_chunks * 8], I16)
            idx23_t = res.tile([128, total_chunks * 8], I16)
            w_t = res.tile([128, total_chunks], BF16)
            dl_t = res.tile([128, total_chunks], BF16)
            iota_t = res.tile([128, 128], BF16)
            ident_t = res.tile([128, 128], BF16)
            ws_t = res.tile([64, 3 * D], BF16)  # W_l at [:, 64l:64l+64] (transposed in)
            nc.sync.dma_start(idx1_t[:], idx1_d.ap())
            nc.sync.dma_start(idx23_t[:], idx23_d.ap())
            nc.sync.dma_start(w_t[:], w_d.ap())
            nc.sync.dma_start(dl_t[:], dl_d.ap())
            nc.sync.dma_start(iota_t[:], iota_d.ap())
            nc.sync.dma_start(ident_t[:], ident_d.ap())
            # ws stored [3*64, 64] row-major; load each W_l as a [64,64] slice
            for l in range(3):
                nc.sync.dma_start(
                    ws_t[:, 64 * l : 64 * l + 64],
                    bass.AP(ws_d, l * 64 * 64, [[64, 64], [1, 64]]))

            qctr = [0]
            sched_v = sched["v_counts"]

            # pre-zero the gather buffer slots: partial-tail gathers leave
            # stale bytes which the S mask zeros via w=0 — must be finite.
            for _z in range(12):
                zt = gpool.tile([128, MAX_CALL_CHUNKS, 128], BF16, tag="g")
                nc.vector.memset(zt[:], 0.0)

            def gather_calls(table_ap_A, table_ap_B, idx_t, layer_tag):
                """Emit gathers + S builds + matmuls for all groups of a layer.

                Returns list over groups of the psum tile holding agg [128,64] f32.
                """
                aggs = []
                ci = 0
                v_counts = sched_v
                for g in range(n_groups):
                    agg = ps_agg.tile([128, D], F32, tag="agg")
                    first = True
                    cc = ci  # running chunk cursor across both halves
                    for h in range(2):
                        k = int(n_chunks[g, h])
                        if k == 0:
                            continue
                        vcnt = int(v_counts[g, h])
                        tab_ap = table_ap_A if h == 0 else table_ap_B
                        done = 0
                        while done < k:
                            nk = min(MAX_CALL_CHUNKS, k - done)
                            ni = min(nk * 128, vcnt - done * 128)
                            gt = gpool.tile([128, nk, 128], BF16, tag="g")
                            nc.gpsimd.dma_gather(
                                gt[:],
                                tab_ap,
                                idx_t[:, (cc + done) * 8 : (cc + done) * 8 + ni // 16],
                                ni, ni, 128,
                                single_packet=False,
                                queue_num=qctr[0] % NQ,
                            )
                            qctr[0] += 1
                            st = spool.tile([128, nk, 128], BF16, tag="s")
                            iota_b = bass.AP(iota_t.tensor, iota_t.offset,
                                             [iota_t[:].ap[0], [0, nk], [1, 128]])
                            dlsl = dl_t[:, cc + done : cc + done + nk]
                            dl_b = bass.AP(dlsl.tensor, dlsl.offset,
                                           [dlsl.ap[0], [1, nk], [0, 128]])
                            wsl = w_t[:, cc + done : cc + done + nk]
                            w_b = bass.AP(wsl.tensor, wsl.offset,
                                          [wsl.ap[0], [1, nk], [0, 128]])
                            nc.vector.tensor_tensor(
                                st[:], iota_b, dl_b, op=mybir.AluOpType.is_equal)
                            nc.vector.tensor_tensor(
                                st[:], st[:], w_b, op=mybir.AluOpType.mult)
                            for j in range(nk):
                                is_last = (done + j == k - 1) and (
                                    h == 1 or n_chunks[g, 1] == 0)
                                nc.tensor.matmul(
                                    agg[:], st[:, j, :], gt[:, j, 0:D],
                                    start=first, stop=is_last)
                                first = False
                            done += nk
                        cc += k
                    ci += int(n_chunks[g].sum())
                    aggs.append(agg)
                assert ci == total_chunks
                return aggs

            def table_view(t_d, nrows, offB):
                va = min(32768, nrows)
                apA = bass.AP(t_d, 0, [[2 * D, va], [1, 2 * D]])
                apB = bass.AP(t_d, offB * 2 * D, [[2 * D, va], [1, 2 * D]])
                return apA, apB

            def epilogue_to_table(g, agg, w_slice, dst_loc):
                """h = LR(agg) (layer>=2 path); write (h @ W_next) dup'd."""
                h_sb = epool.tile([128, D], BF16, tag="h")
                nc.scalar.activation(h_sb[:], agg[:],
                                     mybir.ActivationFunctionType.Lrelu,
                                     alpha=NEG_SLOPE)
                trp = ps_tr.tile([64, 128], BF16, tag="tr")
                nc.tensor.transpose(trp[:], h_sb[:], ident_t[:])
                trs = epool.tile([64, 128], BF16, tag="trs")
                nc.scalar.copy(trs[:], trp[:])
                tp = ps_w.tile([128, D], F32, tag="tw")
                nc.tensor.matmul(tp[:], trs[:], w_slice, start=True, stop=True)
                ts = epool.tile([128, 2 * D], BF16, tag="ts")
                nc.vector.tensor_copy(ts[:, 0:D], tp[:])
                nc.vector.tensor_copy(ts[:, D : 2 * D], tp[:])
                nc.sync.dma_start(
                    bass.AP(dst_loc, g * GROUP * 2 * D, [[2 * D, 128], [1, 2 * D]]),
                    ts[:])

            # ================= layer 1 =================
            apA, apB = table_view(xdup_d, n_nodes, offB_l1)
            aggs = gather_calls(apA, apB, idx1_t, "l1")
            for g, agg in enumerate(aggs):
                # h1 = LR(agg @ W1); table2 = h1 @ W2
                a_sb = epool.tile([128, D], BF16, tag="h")
                nc.vector.tensor_copy(a_sb[:], agg[:])
                trp = ps_tr.tile([64, 128], BF16, tag="tr")
                nc.tensor.transpose(trp[:], a_sb[:], ident_t[:])
                trs = epool.tile([64, 128], BF16, tag="trs")
                nc.scalar.copy(trs[:], trp[:])
                # (agg@W1)^T = W1^T @ agg^T : lhsT = W1 (stored [k,n] row-major)
                gp = ps_w.tile([64, 128], F32, tag="tw")
                nc.tensor.matmul(gp[:], ws_t[:, 0:D], trs[:], start=True, stop=True)
                h1T = epool.tile([64, 128], BF16, tag="h1t")
                nc.scalar.activation(h1T[:], gp[:],
                                     mybir.ActivationFunctionType.Lrelu,
                                     alpha=NEG_SLOPE)
                t2p = ps_w.tile([128, D], F32, tag="tw")
                nc.tensor.matmul(t2p[:], h1T[:], ws_t[:, D : 2 * D],
                                 start=True, stop=True)
                t2s = epool.tile([128, 2 * D], BF16, tag="ts")
                nc.vector.tensor_copy(t2s[:, 0:D], t2p[:])
                nc.vector.tensor_copy(t2s[:, D : 2 * D], t2p[:])
                nc.sync.dma_start(
                    bass.AP(g2_loc, g * GROUP * 2 * D, [[2 * D, 128], [1, 2 * D]]),
                    t2s[:])

            nc.gpsimd.collective_compute(
                "AllGather", mybir.AluOpType.bypass,
                replica_groups=[list(range(NC))],
                ins=[g2_loc.ap().opt()], outs=[g2_full.ap().opt()])

            # ================= layer 2 =================
            apA, apB = table_view(g2_full, tab_rows, offB_l23)
            aggs = gather_calls(apA, apB, idx23_t, "l2")
            for g, agg in enumerate(aggs):
                epilogue_to_table(g, agg, ws_t[:, 2 * D : 3 * D], g3_loc)

            nc.gpsimd.collective_compute(
                "AllGather", mybir.AluOpType.bypass,
                replica_groups=[list(range(NC))],
                ins=[g3_loc.ap().opt()], outs=[g3_full.ap().opt()])

            if os.environ.get("KERNEL_DEBUG"):
                dbg2_d = nc.dram_tensor("dbg2", [rows_pad, 2 * D], BF16,
                                        kind="ExternalOutput")
                dbg3_d = nc.dram_tensor("dbg3", [rows_pad, 2 * D], BF16,
                                        kind="ExternalOutput")
                nc.sync.dma_start(dbg2_d.ap(), g2_loc.ap())
                nc.sync.dma_start(dbg3_d.ap(), g3_loc.ap())

            # ================= layer 3 =================
            apA, apB = table_view(g3_full, tab_rows, offB_l23)
            aggs = gather_calls(apA, apB, idx23_t, "l3")
            for g, agg in enumerate(aggs):
                o_sb = epool.tile([128, D], F32, tag="o")
                nc.scalar.activation(o_sb[:], agg[:],
                                     mybir.ActivationFunctionType.Lrelu,
                                     alpha=NEG_SLOPE)
                rows = min(GROUP, nodes_per_core - g * GROUP)
                nc.sync.dma_start(
                    bass.AP(out_d, g * GROUP * D, [[D, rows], [1, D]]),
                    o_sb[0:rows, :])

    nc.compile()
    return nc


def _run(x, edge_index, edge_w, W1, W2, W3):
    global LAST_EXEC_NS
    n_nodes = x.shape[0]
    nodes_per_core = n_nodes // NC
    src = np.asarray(edge_index[0], np.int64)
    dst = np.asarray(edge_index[1], np.int64)
    w = np.asarray(edge_w, np.float32)

    n_groups = int(np.ceil(nodes_per_core / GROUP))
    rows_pad = n_groups * GROUP
    sched, per_core = _prep_edges(src, dst, w, n_nodes, nodes_per_core,
                                  tab_rows_l1=n_nodes, tab_rows_l23=NC * rows_pad)

    nc = _build_nc(n_nodes, sched)

    xdup = np.concatenate([x, x], axis=1).astype(ml_dtypes.bfloat16)
    iota = np.tile(np.arange(128, dtype=ml_dtypes.bfloat16)[None, :], (128, 1))
    ident = np.eye(128, dtype=ml_dtypes.bfloat16)
    ws = np.concatenate([np.asarray(Wi, np.float32) for Wi in (W1, W2, W3)],
                        axis=0).astype(ml_dtypes.bfloat16)

    in_maps = []
    for c in range(NC):
        pc = per_core[c]
        in_maps.append({
            "xdup": xdup,
            "idx1": _wrap_idx(pc["idx1"]),
            "idx23": _wrap_idx(pc["idx23"]),
            "wv": pc["w"].T.astype(ml_dtypes.bfloat16).copy(),
            "dl": pc["dl"].T.astype(ml_dtypes.bfloat16).copy(),
            "iota": iota,
            "ident": ident,
            "ws": ws,
        })

    trace = bool(int(os.environ.get("KERNEL_PROFILE", "0")))
    if trace:
        _install_profile_shim()
    res = run_bass_kernel_spmd(nc, in_maps, core_ids=list(range(NC)), trace=trace)
    LAST_EXEC_NS = res.exec_time_ns
    globals()["LAST_RESULTS"] = res.results
    out = np.concatenate([res.results[c]["out"] for c in range(NC)], axis=0)
    return out.astype(np.float32)


def kernel(x, edge_index, edge_w, W1, W2, W3):
    x = np.asarray(x, np.float32)
    assert x.shape == (50000, 64)
    return _run(x, np.asarray(edge_index), np.asarray(edge_w), W1, W2, W3)

